# revision 1
# baseline (speedup 1.0000x reference)
"""EEGGraphConvNetLSTM on 8 TRN2 NeuronCores (Bass/Tile).

Strategy: graph-level data parallel. Each core gets 16 graphs (1024 nodes)
plus a 64-node halo (previous core's last graph) used to burn in the LSTM
state. GCN message passing is done as dense block-diagonal [128x128]
adjacency matmuls (2 graphs per block). BatchNorm batch statistics are
all-reduced across cores. The 8192-step LSTM is run as 128 parallel chunks
of 8 steps per core, each chunk warmed up with B=32 burn-in steps (forget-
gate decay makes the truncation error ~1e-3).
"""

import numpy as np
from contextlib import ExitStack

import concourse.bass as bass
import concourse.mybir as mybir
from concourse.tile import TileContext
from concourse.bass_utils import run_bass_kernel_spmd
from concourse.vector_clock import ScopedClock

# ---------------- walrus workaround: <=1 sync wait per instruction ----------
import concourse.tile as tile_mod


def _split_all_waits(nc):
    for _, b in list(nc.bb_map.items()):
        insts = b.bb.instructions
        out = []
        changed = False
        for ins in insts:
            si = getattr(ins, "sync_info", None)
            if si is not None and si.on_wait and len(si.on_wait) > 1:
                waits = list(si.on_wait)
                spill, keep = waits[:-1], waits[-1:]
                si.on_wait = keep
                for w in spill:
                    nop = mybir.InstNoOp(
                        name=nc.get_next_instruction_name(), ins=[], outs=[]
                    )
                    nop.engine = ins.engine
                    nop.sync_info = mybir.SyncInfo(on_wait=[w], on_update=[])
                    nc.register_instruction(nop)
                    out.append(nop)
                changed = True
            out.append(ins)
        if changed:
            b.bb.instructions[:] = out


def _patched_drain(self, tick_clock, wait_clock):
    nc = self.nc
    drain = nc.sync.drain()
    wait_clock.add_sem_waits(drain.ins, ScopedClock({None: tick_clock.global_clock}))
    nc.all_engine_barrier()
    assert self.sems is not None
    popped = nc._tile_sem_poison_stack.pop()
    assert popped is self._sem_poison
    nc.clear_and_free_semaphores(list(self.sems.allocated().values()))
    nc.all_engine_barrier()
    _split_all_waits(nc)


tile_mod.TileContext._drain_and_barrier = _patched_drain

# ---------------- constants ----------------
NCORES = 8
G, NPG = 128, 64          # graphs, nodes per graph
GPC = G // NCORES         # 16 graphs per core
NLOC = GPC * NPG          # 1024 own nodes
PAD = 64                  # halo (prev graph) + tail zero pad
NT = NLOC + 2 * PAD       # 1152 node columns per core
NB = NT // 128            # 9 two-graph blocks
LCH = 8                   # chunk length
C = 128                   # chunks per core
BURN = 24                 # LSTM burn-in steps
STEPS = BURN + LCH
H = 256
N_NODES = 8192

DT32 = mybir.dt.float32
DT32R = mybir.dt.float32r
DT16 = mybir.dt.float16
AF = mybir.ActivationFunctionType
ALU = mybir.AluOpType

LAYERS = [(1280, 640), (640, 512), (512, 256)]

_CACHE = {}


def _build():
    nc = bass.Bass()
    # ---- dram params (fp16 for everything feeding fp16 matmuls)
    xT = nc.declare_dram_parameter("xT", [1280, NT], DT16, isOutput=False)
    WT = [
        nc.declare_dram_parameter(f"WT{l+1}", [fi, fo], DT16, isOutput=False)
        for l, (fi, fo) in enumerate(LAYERS)
    ]
    AT = nc.declare_dram_parameter("AT", [NB, 128, 128], DT16, isOutput=False)
    gv = [nc.declare_dram_parameter(f"g{l+1}", [128, LAYERS[l][1] // 128], DT32, False) for l in range(3)]
    bev = [nc.declare_dram_parameter(f"be{l+1}", [128, LAYERS[l][1] // 128], DT32, False) for l in range(3)]
    WihT = nc.declare_dram_parameter("WihT", [256, 1024], DT16, isOutput=False)
    bihh = nc.declare_dram_parameter("bihh", [128, 8], DT32, isOutput=False)
    WhhT = nc.declare_dram_parameter("WhhT", [256, 1024], DT16, isOutput=False)
    ident = nc.declare_dram_parameter("ident", [128, 128], DT16, isOutput=False)
    masks = nc.declare_dram_parameter("masks", [4, 128, 2 * C], DT32, isOutput=False)
    fW1T = nc.declare_dram_parameter("fW1T", [256, 128], DT32, isOutput=False)
    fW2T = nc.declare_dram_parameter("fW2T", [128, 64], DT32, isOutput=False)
    fW3T = nc.declare_dram_parameter("fW3T", [64, 2], DT32, isOutput=False)
    fb1 = nc.declare_dram_parameter("fb1", [128, 1], DT32, isOutput=False)
    fb2 = nc.declare_dram_parameter("fb2", [64, 1], DT32, isOutput=False)
    fb3 = nc.declare_dram_parameter("fb3", [2, 1], DT32, isOutput=False)
    out_d = nc.declare_dram_parameter("out", [2, GPC], DT32, isOutput=True)

    cc_in = [nc.dram_tensor(f"cc_in{l}", [128, 2 * (LAYERS[l][1] // 128)], DT32) for l in range(3)]
    cc_out = [
        nc.dram_tensor(f"cc_out{l}", [128, 2 * (LAYERS[l][1] // 128)], DT32, addr_space="Shared")
        for l in range(3)
    ]
    rg = [list(range(NCORES))]
    cc_wi = nc.dram_tensor("cc_wi", [128, 1], DT32)
    cc_wo = nc.dram_tensor("cc_wo", [128, 1], DT32, addr_space="Shared")

    with TileContext(nc) as tc, ExitStack() as ctx:
        wp = ctx.enter_context(tc.tile_pool(name="wp", bufs=1))
        big = ctx.enter_context(tc.tile_pool(name="big", bufs=1))

        # ---- persistent weight/const tiles
        def load2d(dram, rows, cols, dt, tag, r0=0, c0=0):
            t = wp.tile([rows, cols], dt, tag=tag)
            nc.sync.dma_start(out=t[:], in_=dram[r0 : r0 + rows, c0 : c0 + cols])
            return t

        warm = wp.tile([128, 1], DT32, tag="warm", name="warm")
        nc.vector.memset(warm[:], 0.0)
        nc.sync.dma_start(out=cc_wi[:], in_=warm[:])
        nc.gpsimd.collective_compute(
            "AllReduce", ALU.add, replica_groups=rg, ins=[cc_wi[:]], outs=[cc_wo[:]])
        xTt = [load2d(xT, 128, NT, DT16, f"xT{k}", r0=k * 128) for k in range(10)]
        WTt = []
        for l, (fi, fo) in enumerate(LAYERS):
            WTt.append([load2d(WT[l], 128, fo, DT16, f"WT{l}_{k}", r0=k * 128) for k in range(fi // 128)])
        ATt = []
        for b in range(NB):
            t = wp.tile([128, 128], DT16, tag=f"AT{b}", name=f"AT{b}")
            nc.sync.dma_start(out=t[:], in_=AT[b, :, :])
            ATt.append(t)
        WihTt = [[load2d(WihT, 128, 128, DT16, f"WihT{k}_{m}", r0=k * 128, c0=m * 128) for m in range(8)] for k in range(2)]
        WhhTt = [[load2d(WhhT, 128, 128, DT16, f"WhhT{k}_{m}", r0=k * 128, c0=m * 128) for m in range(8)] for k in range(2)]
        idt = load2d(ident, 128, 128, DT16, "ident")
        # per-feature g/be as [128, nft]
        gT, beT = [], []
        for l, (fi, fo) in enumerate(LAYERS):
            nft = fo // 128
            tg = wp.tile([128, nft], DT32, tag=f"gT{l}", name=f"gT{l}")
            tb = wp.tile([128, nft], DT32, tag=f"beT{l}", name=f"beT{l}")
            nc.sync.dma_start(out=tg[:], in_=gv[l][:, :])
            nc.sync.dma_start(out=tb[:], in_=bev[l][:, :])
            gT.append(tg)
            beT.append(tb)
        bihh_t = wp.tile([128, 8], DT32, tag="bihh", name="bihh")
        nc.sync.dma_start(out=bihh_t[:], in_=bihh[:, :])
        msk32, msk16 = [], []
        for i in range(4):
            m32 = wp.tile([128, 2 * C], DT32, tag=f"m32_{i}", name=f"m32_{i}")
            nc.sync.dma_start(out=m32[:], in_=masks[i, :, :])
            m16 = wp.tile([128, 2 * C], DT16, tag=f"m16_{i}", name=f"m16_{i}")
            nc.vector.tensor_copy(m16[:], m32[:])
            msk32.append(m32)
            msk16.append(m16)
        fW1Tt = [load2d(fW1T, 128, 128, DT32, f"fW1T{k}", r0=k * 128) for k in range(2)]
        fW2Tt = load2d(fW2T, 128, 64, DT32, "fW2T")
        fW3Tt = load2d(fW3T, 64, 2, DT32, "fW3T")
        fb1t = wp.tile([128, 1], DT32, tag="fb1", name="fb1")
        nc.sync.dma_start(out=fb1t[:], in_=fb1[:, :])
        fb2t = wp.tile([64, 1], DT32, tag="fb2", name="fb2")
        nc.sync.dma_start(out=fb2t[:], in_=fb2[:, :])
        fb3t = wp.tile([2, 1], DT32, tag="fb3", name="fb3")
        epst = wp.tile([128, 1], DT32, tag="epst", name="epst")
        nc.vector.memset(epst[:], 1e-5)
        nc.sync.dma_start(out=fb3t[:], in_=fb3[:, :])

        # ---------------- GCN layers ----------------
        hT = xTt
        psA_cm = tc.tile_pool(name="psA", bufs=1, space="PSUM")
        psA = psA_cm.__enter__()

        for l, (fi, fo) in enumerate(LAYERS):
            K = fi // 128
            nft = fo // 128
            # lin: m[node, fo] node-major, fp16
            m16t = [big.tile([128, 640], DT16, tag=f"m16_{b}", name=f"m16_{b}") for b in range(NB)]
            for nt in range(NB):
                ps = psA.tile([128, 1024], DT32, tag="linps", name="linps", bufs=2)
                if fo == 640:
                    chunks = [(0, 0, 320), (320, 512, 320)]  # (m-col, psum-col, width)
                elif fo == 512:
                    chunks = [(0, 0, 512)]
                else:
                    chunks = [(0, 0, 256)]
                for k in range(K):
                    for (mc, pc, w) in chunks:
                        nc.tensor.matmul(
                            ps[:, pc : pc + w],
                            lhsT=hT[k][:, nt * 128 : (nt + 1) * 128],
                            rhs=WTt[l][k][:, mc : mc + w],
                            start=(k == 0),
                            stop=(k == K - 1),
                        )
                for (mc, pc, w) in chunks:
                    nc.vector.tensor_copy(m16t[nt][:, mc : mc + w], ps[:, pc : pc + w])
            # scatter: s.T[f, dst] feature-major fp32 + stats
            sT = [big.tile([128, NT], DT32, tag=f"sT{ft}", name=f"sT{ft}") for ft in range(nft)]
            stats = big.tile([128, 2 * nft], DT32, tag=f"stats{l}", name=f"stats{l}")
            sqs = big.tile([128, NLOC], DT32, tag="sqscratch", name="sqscratch")
            for ft in range(nft):
                pss = psA.tile([128, NT], DT32, tag="scps", name="scps")
                for b in range(NB):
                    nc.tensor.matmul(
                        pss[:, b * 128 : (b + 1) * 128],
                        lhsT=m16t[b][:, ft * 128 : (ft + 1) * 128],
                        rhs=ATt[b][:],
                        start=(b % 4 == 0),
                        stop=(b in (3, 7, 8)),
                    )
                nc.scalar.activation(sT[ft][:, 0:PAD], pss[:, 0:PAD], AF.Copy)
                nc.scalar.activation(
                    sT[ft][:, PAD:NT], pss[:, PAD:NT], AF.Copy,
                    accum_out=stats[:, ft : ft + 1],
                )
            for ft in range(nft):
                nc.scalar.activation(
                    sqs[:], sT[ft][:, PAD : PAD + NLOC], AF.Square,
                    accum_out=stats[:, nft + ft : nft + ft + 1],
                )
            # allreduce stats
            nc.sync.dma_start(out=cc_in[l][:], in_=stats[:])
            nc.gpsimd.collective_compute(
                "AllReduce", ALU.add, replica_groups=rg,
                ins=[cc_in[l][:]], outs=[cc_out[l][:]],
            )
            statsg = big.tile([128, 2 * nft], DT32, tag=f"statsg{l}", name=f"statsg{l}")
            nc.sync.dma_start(out=statsg[:], in_=cc_out[l][:])
            # scale/bias
            mu = big.tile([128, nft], DT32, tag="mu", name="mu")
            var = big.tile([128, nft], DT32, tag="var", name="var")
            scl = big.tile([128, nft], DT32, tag="scl", name="scl")
            bia = big.tile([128, nft], DT32, tag="bia", name="bia")
            nc.vector.tensor_scalar_mul(mu[:], statsg[:, 0:nft], 1.0 / N_NODES)
            nc.vector.tensor_scalar_mul(var[:], statsg[:, nft : 2 * nft], 1.0 / N_NODES)
            nc.vector.tensor_mul(scl[:], mu[:], mu[:])
            nc.vector.tensor_sub(var[:], var[:], scl[:])
            nc.scalar.activation(var[:], var[:], AF.Sqrt, bias=epst[:])
            nc.vector.reciprocal(var[:], var[:])
            nc.vector.tensor_mul(scl[:], gT[l][:], var[:])
            nc.vector.tensor_mul(mu[:], mu[:], scl[:])
            nc.vector.tensor_sub(bia[:], beT[l][:], mu[:])
            # apply + leaky -> next hT (fp16, feature-major)
            hTn = [big.tile([128, NT], DT16, tag=f"hT{l}_{ft}", name=f"hT{l}_{ft}") for ft in range(nft)]
            for ft in range(nft):
                nc.scalar.activation(
                    hTn[ft][:], sT[ft][:], AF.Lrelu,
                    bias=bia[:, ft : ft + 1], scale=scl[:, ft : ft + 1], alpha=0.01,
                )
            hT = hTn

        # ---------------- pre-gates: PreT[m] = [gate, node] fp16 ----------------
        PreT = [big.tile([128, NT], DT16, tag=f"PreT{m}", name=f"PreT{m}") for m in range(8)]
        for m in range(8):
            for (n0, w) in [(0, 512), (512, 512), (1024, 128)]:
                psp = psA.tile([128, 512], DT32, tag="preps", name="preps")
                for k in range(2):
                    nc.tensor.matmul(
                        psp[:, 0:w],
                        lhsT=WihTt[k][m][:],
                        rhs=hT[k][:, n0 : n0 + w],
                        start=(k == 0),
                        stop=(k == 1),
                    )
                nc.vector.tensor_scalar_add(PreT[m][:, n0 : n0 + w], psp[:, 0:w], bihh_t[:, m : m + 1])

        psA_cm.__exit__(None, None, None)

        # ---------------- LSTM ----------------
        lsp = ctx.enter_context(tc.tile_pool(name="lsp", bufs=2))
        one = ctx.enter_context(tc.tile_pool(name="one", bufs=1))
        h_sb = one.tile([128, 2 * C], DT16, tag="h_sb", name="h_sb")
        c_sb = one.tile([128, 2 * C], DT32, tag="c_sb", name="c_sb")
        acc = one.tile([128, 2 * C], DT32, tag="acc", name="acc")
        nc.vector.memset(h_sb[:], 0.0)
        nc.vector.memset(c_sb[:], 0.0)
        nc.vector.memset(acc[:], 0.0)
        psB = ctx.enter_context(tc.tile_pool(name="psB", bufs=2, space="PSUM"))
        mask_steps = {BURN - 1 - c * LCH: (BURN - 1 - c * LCH - (LCH - 1)) // LCH for c in range(4)}
        # mask index i corresponds to step 7+8i zeroing chunk (BURN-1-t)//LCH
        for t in range(STEPS):
            gps = psB.tile([128, 1024], DT32, tag="gps", name="gps")
            off = PAD - BURN + t
            for m in range(8):
                nc.tensor.matmul(
                    gps[:, m * 128 : (m + 1) * 128],
                    lhsT=idt[:],
                    rhs=PreT[m][:, off : off + C * LCH : LCH],
                    start=(m % 4 == 0),
                    stop=False,
                )
            sg = lsp.tile([128, 1024], DT32, tag="sg", name="sg")
            for m in range(8):
                for k in range(2):
                    nc.tensor.matmul(
                        gps[:, m * 128 : (m + 1) * 128],
                        lhsT=WhhTt[k][m][:],
                        rhs=h_sb[:, k * C : (k + 1) * C],
                        start=False,
                        stop=(k == 1),
                    )
                if m == 3:
                    nc.scalar.activation(sg[:, 0:512], gps[:, 0:512], AF.Sigmoid)
                if m == 5:
                    nc.scalar.activation(sg[:, 512:768], gps[:, 512:768], AF.Sigmoid, scale=2.0)
                if m == 7:
                    nc.scalar.activation(sg[:, 768:1024], gps[:, 768:1024], AF.Sigmoid)
            t1 = lsp.tile([128, 256], DT32, tag="t1", name="t1")
            t2 = lsp.tile([128, 256], DT32, tag="t2", name="t2")
            th = lsp.tile([128, 256], DT32, tag="th", name="th")
            nc.vector.tensor_mul(t1[:], sg[:, 256:512], c_sb[:])
            # i*g with g = 2*sg_g - 1:  t2 = (sg_g*2)*i ; c = t1 + t2 - i
            nc.vector.scalar_tensor_tensor(
                t2[:], sg[:, 512:768], 2.0, sg[:, 0:256], ALU.mult, ALU.mult)
            nc.vector.tensor_add(c_sb[:], t1[:], t2[:])
            nc.vector.tensor_sub(c_sb[:], c_sb[:], sg[:, 0:256])
            # tanh(c) = 2*sigmoid(2c) - 1
            nc.scalar.activation(th[:], c_sb[:], AF.Sigmoid, scale=2.0)
            tho = lsp.tile([128, 256], DT32, tag="tho", name="tho")
            nc.vector.scalar_tensor_tensor(
                tho[:], th[:], 2.0, sg[:, 768:1024], ALU.mult, ALU.mult)
            nc.vector.tensor_sub(tho[:], tho[:], sg[:, 768:1024])
            nc.vector.tensor_copy(h_sb[:], tho[:])
            if t >= BURN:
                nc.vector.tensor_add(acc[:], acc[:], tho[:])
            if t in tuple(BURN - 1 - c * LCH for c in range(4) if BURN - 1 - c * LCH >= 0):
                mi = (BURN - 1 - t) // LCH
                nc.vector.tensor_mul(h_sb[:], h_sb[:], msk16[mi][:])
                nc.vector.tensor_mul(c_sb[:], c_sb[:], msk32[mi][:])

        # ---------------- pool + FC ----------------
        poolT = one.tile([128, 2, GPC], DT32, tag="poolT", name="poolT")
        accv = acc[:].rearrange("p (b g j) -> p b g j", b=2, g=GPC, j=LCH)
        nc.vector.tensor_reduce(poolT[:], accv, axis=mybir.AxisListType.X, op=ALU.add)
        fps = psB.tile([128, GPC], DT32, tag="fcps", name="fcps")
        for k in range(2):
            nc.tensor.matmul(fps[:], lhsT=fW1Tt[k][:], rhs=poolT[:, k, :], start=(k == 0), stop=(k == 1))
        fc1 = one.tile([128, GPC], DT32, tag="fc1", name="fc1")
        nc.scalar.activation(fc1[:], fps[:], AF.Lrelu, bias=fb1t[:], alpha=0.01)
        fps2 = psB.tile([64, GPC], DT32, tag="fcps", name="fcps")
        nc.tensor.matmul(fps2[:], lhsT=fW2Tt[:], rhs=fc1[:], start=True, stop=True)
        fc2 = one.tile([64, GPC], DT32, tag="fc2", name="fc2")
        nc.scalar.activation(fc2[:], fps2[:], AF.Lrelu, bias=fb2t[:], alpha=0.01)
        fps3 = psB.tile([2, GPC], DT32, tag="fcps", name="fcps")
        nc.tensor.matmul(fps3[:], lhsT=fW3Tt[:], rhs=fc2[:], start=True, stop=True)
        fc3 = one.tile([2, GPC], DT32, tag="fc3", name="fc3")
        nc.scalar.activation(fc3[:], fps3[:], AF.Lrelu, bias=fb3t[:], alpha=0.01)
        nc.sync.dma_start(out=out_d[:], in_=fc3[:])

    return nc


def _prep_core(inputs, k, A):
    f16 = np.float16
    x = inputs["x"]
    lo, hi = k * NLOC - PAD, k * NLOC + NLOC
    xTk = np.zeros((1280, NT), f16)
    if k == 0:
        xTk[:, PAD : PAD + NLOC] = x[0:NLOC].T
    else:
        xTk[:, 0 : PAD + NLOC] = x[lo:hi].T
    ATk = np.zeros((NB, 128, 128), f16)
    glist = ([-1] if k == 0 else [k * GPC - 1]) + list(range(k * GPC, (k + 1) * GPC)) + [-1]
    for b in range(NB):
        ga, gb = glist[2 * b], glist[2 * b + 1]
        if ga >= 0:
            ATk[b, 0:64, 0:64] = A[ga].T
        if gb >= 0:
            ATk[b, 64:128, 64:128] = A[gb].T
    mk = np.ones((4, 2 * C), np.float32)
    if k == 0:
        for c in range(4):
            if BURN - 1 - c * LCH >= 0:
                mk[c, c] = 0.0
                mk[c, C + c] = 0.0
    im = {
        "xT": xTk,
        "WT1": inputs["W1"].T.astype(f16).copy(),
        "WT2": inputs["W2"].T.astype(f16).copy(),
        "WT3": inputs["W3"].T.astype(f16).copy(),
        "AT": ATk,
        "g1": inputs["g1"].astype(np.float32).reshape(5, 128).T.copy(),
        "g2": inputs["g2"].astype(np.float32).reshape(4, 128).T.copy(),
        "g3": inputs["g3"].astype(np.float32).reshape(2, 128).T.copy(),
        "be1": inputs["be1"].astype(np.float32).reshape(5, 128).T.copy(),
        "be2": inputs["be2"].astype(np.float32).reshape(4, 128).T.copy(),
        "be3": inputs["be3"].astype(np.float32).reshape(2, 128).T.copy(),
        "WihT": inputs["Wih"].T.astype(f16).copy(),
        "bihh": (inputs["bih"] + inputs["bhh"]).astype(np.float32).reshape(8, 128).T.copy(),
        "WhhT": inputs["Whh"].T.astype(f16).copy(),
        "ident": np.eye(128, dtype=f16),
        "masks": np.repeat(mk[:, None, :], 128, axis=1),
        "fW1T": inputs["fW1"].T.astype(np.float32).copy(),
        "fW2T": inputs["fW2"].T.astype(np.float32).copy(),
        "fW3T": inputs["fW3"].T.astype(np.float32).copy(),
        "fb1": inputs["fb1"].astype(np.float32).reshape(128, 1),
        "fb2": inputs["fb2"].astype(np.float32).reshape(64, 1),
        "fb3": inputs["fb3"].astype(np.float32).reshape(2, 1),
    }
    return im


def kernel(**inputs):
    inputs = {k: np.asarray(v) for k, v in inputs.items()}
    src, dst = inputs["edge_index"][0], inputs["edge_index"][1]
    ew = inputs["edge_weight"].astype(np.float32)
    A = np.zeros((G, NPG, NPG), np.float32)
    np.add.at(A, (src // NPG, dst % NPG, src % NPG), ew)
    if "nc" not in _CACHE:
        _CACHE["nc"] = _build()
    nc = _CACHE["nc"]
    in_maps = [_prep_core(inputs, k, A) for k in range(NCORES)]
    res = run_bass_kernel_spmd(nc, in_maps, core_ids=list(range(NCORES)), **_CACHE.get("kw", {}))
    _CACHE["last"] = res
    out = np.zeros((G, 2), np.float32)
    for k in range(NCORES):
        out[k * GPC : (k + 1) * GPC, :] = res.results[k]["out"].T
    return out

